# revision 51
# baseline (speedup 1.0000x reference)
"""Trainium2 Bass kernel for nn_MultiHeadAttention_41455024341166.

Reference computation (B=4, S=2048, M=2048, H=16, D=128, fp32):
    qkv = einsum('bsm,mthd->bsthd', x, Wqkv); q,k,v = qkv[:,:,0..2]
    q,k = rope_consecutive(q), rope_consecutive(k)
    ctx = causal_softmax(q @ k^T / sqrt(D)) @ v   (per b,h)
    out = ctx.reshape(B,S,H*D) @ Wo

Sharding: 8 cores = 4 batches x 2 head-groups (core c -> b=c//2, g=c%2,
heads [8g, 8g+8)). Attention is fully head-parallel. For the output
projection each core owns M-columns [g*1024,(g+1)*1024): after each query
strip is normalized, the strip's ctxT (1MB bf16) is AllGathered within the
batch pair so both cores hold all 16 heads' context and project their own
column half with no cross-core reduction (4MB wire per core total, ~20us
per gather, overlapped with later strips' attention).

Kernel strategy (per core; all matmul operands bf16 - fp32r is full-rate
on the PE but its full-width multiplies draw enough power to trip the
activity throttle (50% util cap); bf16 runs measurably cooler. PSUM
accumulation is fp32 throughout; rel err vs the fp32 reference ~6e-3):
  A:  xT resident in SBUF once (per-block DMAs so compute starts early).
      A-qk: qT,kT = W^T-stationary @ xT-moving -> [d, s] layout; RoPE via a
            pair-swap permutation matmul + elementwise cos/sin tables, with
            the RoPE tail software-pipelined one tile behind the projection
            so the PE never waits on the scalar-engine PSUM evacuation.
      A-v:  v = xT-stationary @ Wv-moving -> [s, d], evacuated straight
            into a persistent SBUF tile (v never touches DRAM).
  B:  per head, per 512-query strip, two passes, loads prefetched one head
      ahead:
      pass1: scoresT[j,i] = krotT_j-stationary @ qrotT-moving (transposed
             scores - no prob transpose needed), diagonal blocks sliced to
             the causal region; exp fused into the PSUM evacuation (no max
             subtraction; scores are O(5) here); causal mask =
             multiplicative 0/1 mask after exp (on DVE); softmax
             denominators accumulate via ones-vector matmuls.
      pass2: ctxT += v_j-stationary @ expT-moving; each head's final
             accumulation step + denominator handoff is deferred into the
             next head's body so the new head's first exp overlaps it. The
             [1,512] reciprocal (3.3us, single-lane on DVE) runs early and
             the normalization (a K=1 ones broadcast matmul + DVE mul) is
             deferred ~4 score blocks so it never stalls the PE.
  C:  per strip, after its pairwise ctx AllGather lands: own output
      columns = gathered-ctxT-stationary @ Wo-moving contracted over all
      16 heads; Wo (4MB bf16, all heads x own columns) is SBUF-resident
      from kernel start. Each strip's gather is split in halves (heads 0-3
      fire mid-strip, heads 4-7 ride the next strip, except strip 3's at
      its end) and C chunks are scheduled mid-later-strips, so only the
      last strip's half-gather + C chunk (~40us) is exposed at the tail.
      DMA descriptor writes ride the otherwise-idle gpsimd/sync queues --
      a dma_start occupies its issuing engine ~0.6-2us, which would starve
      the ACT exp pipeline (B's pacer) or the evacuations in A.
"""

import os
import sys
import types
import math

import ml_dtypes
import numpy as np

import concourse.bass as bass
import concourse.tile as tile
import concourse.mybir as mybir
from concourse.bass_utils import run_bass_kernel_spmd

F32 = mybir.dt.float32
F32R = mybir.dt.float32r
BF16 = mybir.dt.bfloat16

B, S, M, H, D = 4, 2048, 2048, 16, 128
HL = H // 2              # heads per core
HD = HL * D              # 1024
SCALE = 1.0 / math.sqrt(D)
MIN_WINDOW, MAX_WINDOW = 1.0, 10000.0

DEBUG = os.environ.get("MHA_KERNEL_DEBUG", "0") == "1"


# ---------------------------------------------------------------------------
# Workarounds for the trimmed walrus/axon stack in this container.
# ---------------------------------------------------------------------------

_WSPLIT_N = [0]


def _split_excess_waits(nc):
    """walrus here rejects instructions carrying more sync-waits than slots
    (1; EventSemaphore: 2). Hoist excess waits onto EventSemaphore carriers
    inserted before the offender on the same engine stream. Safe: Tile emits
    one linearized order where every wait's producer precedes its consumer."""
    for fn in nc.m.functions:
        for bb in fn.blocks:
            changed = False
            new_list = []
            for inst in bb.instructions:
                si = inst.sync_info
                waits = list(si.on_wait) if si is not None else []
                cap = 2 if isinstance(inst, mybir.InstEventSemaphore) else 1
                if len(waits) > cap:
                    keep, excess = waits[-cap:], waits[:-cap]
                    for i in range(0, len(excess), 2):
                        _WSPLIT_N[0] += 1
                        new_list.append(mybir.InstEventSemaphore(
                            name=f"wsplit-{_WSPLIT_N[0]}", ins=[], outs=[],
                            engine=inst.engine,
                            sync_info=mybir.SyncInfo(on_wait=excess[i:i + 2],
                                                     on_update=[])))
                    si.on_wait = keep
                    changed = True
                new_list.append(inst)
            if changed:
                bb.instructions = new_list


def _register_ntff_hook():
    """antenv.axon_hooks is absent in this image, so boot skipped registering
    the NTFF profiling hook; recreate it so trace=True works."""
    if "antenv.axon_hooks" in sys.modules:
        return
    try:
        import antenv as _antenv
        m = types.ModuleType("antenv.axon_hooks")
        m._hook = None
        m.set_axon_ntff_profile_hook = lambda h, _m=m: setattr(_m, "_hook", h)
        m.get_axon_ntff_profile_hook = lambda _m=m: _m._hook
        sys.modules["antenv.axon_hooks"] = m
        _antenv.axon_hooks = m
        from trn_agent_boot.trn_boot import _ntff_profile_via_ctypes
        m.set_axon_ntff_profile_hook(
            _ntff_profile_via_ctypes('/opt/axon/libaxon_pjrt.so'))
    except Exception:
        pass


_register_ntff_hook()


# ---------------------------------------------------------------------------
# Kernel builder (per-core SPMD program)
# ---------------------------------------------------------------------------

def _blocked_dma(eng, dst_ap, dram_full, c0, c1, nrows=None):
    """One DMA moving cols [c0,c1) (and optionally only the first nrows rows)
    of a [R, C] DRAM tensor into a [128, (nrows//128)*(c1-c0)] SBUF tile whose
    column block a holds source rows [a*128, (a+1)*128)."""
    src = dram_full.rearrange("(a p) c -> p a c", p=128)
    if nrows is not None:
        src = src[:, 0:nrows // 128, :]
    src = src[:, :, c0:c1]
    dst = dst_ap.rearrange("p (a c) -> p a c", c=c1 - c0)
    eng.dma_start(dst, src)


def build_kernel():
    nc = bass.Bass("TRN2", target_bir_lowering=False, num_devices=8)

    xt = nc.dram_tensor("xt", [M, S], BF16, kind="ExternalInput")       # x[b].T
    wq = nc.dram_tensor("wq", [M, HD], BF16, kind="ExternalInput")
    wk = nc.dram_tensor("wk", [M, HD], BF16, kind="ExternalInput")
    wv = nc.dram_tensor("wv", [M, HD], BF16, kind="ExternalInput")
    wo = nc.dram_tensor("wo", [2 * HD, M // 2], BF16, kind="ExternalInput")
    cosT = nc.dram_tensor("cosT", [D, S], F32, kind="ExternalInput")
    sinT = nc.dram_tensor("sinT", [D, S], F32, kind="ExternalInput")    # sign-folded
    pmat = nc.dram_tensor("pmat", [D, D], BF16, kind="ExternalInput")   # adjacent-pair swap
    mask128 = nc.dram_tensor("mask128", [128, 128], F32R, kind="ExternalInput")
    # Output columns: this core owns M-columns [g*1024, (g+1)*1024) of out[b].
    # The per-strip ctx AllGather (pairwise) gives each core all 16 heads'
    # context, so its column block needs no cross-core reduction.
    y = nc.dram_tensor("y", [S, M // 2], F32, kind="ExternalOutput")

    dbg = {}
    if DEBUG:
        dbg["qrot"] = nc.dram_tensor("dbg_qrot", [HD, S], BF16, kind="ExternalOutput")
        dbg["krot"] = nc.dram_tensor("dbg_krot", [HD, S], BF16, kind="ExternalOutput")
        dbg["v"] = nc.dram_tensor("dbg_v", [S, HD], BF16, kind="ExternalOutput")
        dbg["ctxT"] = nc.dram_tensor("dbg_ctxT", [HD, S], BF16, kind="ExternalOutput")
        dbg["outp"] = nc.dram_tensor("dbg_outp", [S, M // 2], F32, kind="ExternalOutput")

    with nc.allow_low_precision(reason="fp32r matmul kernel"), \
         tile.TileContext(nc) as tc:
        with tc.tile_pool(name="dram", bufs=1, space="DRAM") as dram, \
             tc.tile_pool(name="wo_res", bufs=1) as wrp:
            qrot_d = dram.tile([HD, S], BF16)
            krot_d = dram.tile([HD, S], BF16)

            # v stays in SBUF end-to-end: block sb of the free axis holds
            # v[sb*128:(sb+1)*128, :] i.e. rows (keys) x all 1024 hd cols, so
            # the PV stationary for (head h, key block jt) is
            # v_res[:, jt*1024 + h*128 : jt*1024 + (h+1)*128].
            v_res = wrp.tile([128, 16 * HD], BF16)

            warm_in = dram.tile([1, 128], F32, name="warm_in")
            warm_out = dram.tile([2, 128], F32, name="warm_out")

            # Wo resident in SBUF for the whole kernel: wo holds all 16 heads'
            # rows x this core's 1024 output columns. Block a of the free axis
            # holds rows [a*128,(a+1)*128) (= global head a), so the C-phase
            # moving operand for head ht / col-strip ms is
            # wos_all[:, ht*1024 + ms*512 : ht*1024 + (ms+1)*512].
            wos_all = wrp.tile([128, 16 * (M // 2)], BF16)
            _blocked_dma(nc.scalar, wos_all[:], wo[:], 0, M // 2)

            # ======== Phase A: projections off one resident xT ========
            # xT lives in 16 per-mt tiles so the first projection matmuls can
            # start as soon as the first 1MB row-block lands.
            with tc.tile_pool(name="ax", bufs=1) as xp, \
                 tc.tile_pool(name="avw", bufs=1) as wvp:
                xts = []


                # ---- A-qk: qT,kT + RoPE ----
                with nc.named_scope("A_qk"):
                    with (
                        tc.tile_pool(name="atab", bufs=1) as tabp,
                        tc.tile_pool(name="aw", bufs=3) as wp,
                        tc.tile_pool(name="aps", bufs=3, space="PSUM") as psp,
                        tc.tile_pool(name="aps2", bufs=2, space="PSUM") as psp2,
                        tc.tile_pool(name="at", bufs=3) as tp,
                    ):
                        cos_sb = tabp.tile([128, S], F32)
                        nc.gpsimd.dma_start(cos_sb[:], cosT[:])
                        sin_sb = tabp.tile([128, S], F32)
                        nc.gpsimd.dma_start(sin_sb[:], sinT[:])
                        p_sb = tabp.tile([128, 128], BF16)
                        nc.gpsimd.dma_start(p_sb[:], pmat[:])
                        # warm up the CC path early so the first real gather
                        # doesn't pay the handshake cost (queued behind the
                        # small table loads above).
                        nc.gpsimd.collective_compute(
                            "AllGather", mybir.AluOpType.bypass,
                            replica_groups=[[0, 1], [2, 3], [4, 5], [6, 7]],
                            ins=[warm_in[:]], outs=[warm_out[:]])
                        # first weight blocks go ahead of the 16MB xT load so
                        # the projection can start as soon as quarter 0 lands
                        wblk_pre = {}
                        for h0, qk0, wt0 in ((0, 0, wq), (0, 1, wk), (1, 0, wq)):
                            wb = wp.tile([128, 16 * 128], BF16,
                                         name=f"wblk{h0}{qk0}", tag="wblk")
                            _blocked_dma(nc.sync, wb[:], wt0[:],
                                         h0 * 128, (h0 + 1) * 128)
                            wblk_pre[(h0, qk0)] = wb
                        for q4 in range(4):
                            xti = xp.tile([128, 4 * S], BF16, name=f"xt{q4}")
                            # per-block DMAs so the first projection chain can
                            # start as soon as the first 512KB lands
                            for a in range(4):
                                nc.sync.dma_start(
                                    xti[:, a * S:(a + 1) * S],
                                    xt.rearrange("(a p) c -> p a c", p=128)
                                      [:, q4 * 4 + a, :])
                            xts.append(xti)
                        wvs0 = wvp.tile([128, 16 * 512], BF16,
                                        name="wvs0", tag="wvs")
                        _blocked_dma(nc.sync, wvs0[:], wv[:], 0, 512)

                        # RoPE tail (pmat matmul + cos/sin fold) runs one tile
                        # behind the projection chain so the PE never waits on
                        # the scalar-engine PSUM evacuation it consumes.
                        rope_pend = []

                        def rope_tail():
                            if not rope_pend:
                                return
                            q_sb, h, qk, t, outd = rope_pend.pop()
                            ps2 = psp2.tile([128, 512], F32,
                                            name=f"psw{h}{qk}{t}", tag="psw")
                            nc.tensor.matmul(ps2[:], p_sb[:], q_sb[:],
                                             start=True, stop=True)
                            t2 = tp.tile([128, 512], F32,
                                         name=f"t2{h}{qk}{t}", tag="t2")
                            nc.vector.tensor_mul(t2[:], ps2[:],
                                                 sin_sb[:, t * 512:(t + 1) * 512])
                            t1 = tp.tile([128, 512], F32,
                                         name=f"t1{h}{qk}{t}", tag="t1")
                            nc.vector.tensor_mul(t1[:], q_sb[:],
                                                 cos_sb[:, t * 512:(t + 1) * 512])
                            qr = tp.tile([128, 512], BF16,
                                         name=f"qr{h}{qk}{t}", tag="qr")
                            nc.vector.tensor_add(qr[:], t1[:], t2[:])
                            nc.gpsimd.dma_start(
                                outd[h * 128:(h + 1) * 128,
                                     t * 512:(t + 1) * 512], qr[:])
                            if DEBUG:
                                nc.sync.dma_start(
                                    dbg["qrot" if qk == 0 else "krot"]
                                    [h * 128:(h + 1) * 128,
                                     t * 512:(t + 1) * 512], qr[:])

                        groups = [(h, qk) for h in range(HL) for qk in (0, 1)]
                        wtab = {0: (wq, qrot_d), 1: (wk, krot_d)}

                        def fetch_wblk(h, qk):
                            if (h, qk) in wblk_pre:
                                return wblk_pre.pop((h, qk))
                            wblk = wp.tile([128, 16 * 128], BF16,
                                           name=f"wblk{h}{qk}", tag="wblk")
                            _blocked_dma(nc.sync, wblk[:], wtab[qk][0][:],
                                         h * 128, (h + 1) * 128)
                            return wblk

                        wnext = fetch_wblk(*groups[0])
                        for gi, (h, qk) in enumerate(groups):
                            wblk = wnext
                            if gi + 1 < len(groups):
                                wnext = fetch_wblk(*groups[gi + 1])
                            outd = wtab[qk][1]
                            if True:
                                for t in range(4):
                                    ps = psp.tile([128, 512], F32,
                                                  name=f"psq{h}{qk}{t}", tag="psq")
                                    for mt in range(16):
                                        nc.tensor.matmul(
                                            ps[:],
                                            wblk[:, mt * 128:(mt + 1) * 128],
                                            xts[mt // 4][:, (mt % 4) * S + t * 512:(mt % 4) * S + (t + 1) * 512],
                                            start=(mt == 0), stop=(mt == 15))
                                    q_sb = tp.tile([128, 512], BF16,
                                                   name=f"q{h}{qk}{t}", tag="q")
                                    nc.scalar.copy(q_sb[:], ps[:])
                                    rope_tail()
                                    rope_pend.append((q_sb, h, qk, t, outd))
                        rope_tail()

                # ---- A-v: v = x @ Wv (natural [s, hd]) ----
                with nc.named_scope("A_v"):
                    with (
                        tc.tile_pool(name="avps", bufs=3, space="PSUM") as psp,
                    ):
                        for ds in range(2):
                            wvs = wvs0 if ds == 0 else wvp.tile(
                                [128, 16 * 512], BF16, name=f"wvs{ds}",
                                tag="wvs")
                            if ds != 0:
                                _blocked_dma(nc.sync, wvs[:], wv[:], ds * 512,
                                             (ds + 1) * 512)
                            for sb in range(16):
                                ps = psp.tile([128, 512], F32,
                                              name=f"psv{ds}{sb}", tag="psv")
                                for mt in range(16):
                                    nc.tensor.matmul(
                                        ps[:],
                                        xts[mt // 4][:, (mt % 4) * S + sb * 128:(mt % 4) * S + (sb + 1) * 128],
                                        wvs[:, mt * 512:(mt + 1) * 512],
                                        start=(mt == 0), stop=(mt == 15))
                                nc.scalar.copy(
                                    v_res[:, sb * HD + ds * 512:
                                          sb * HD + (ds + 1) * 512], ps[:])
                                if DEBUG:
                                    nc.sync.dma_start(
                                        dbg["v"][sb * 128:(sb + 1) * 128,
                                                 ds * 512:(ds + 1) * 512],
                                        v_res[:, sb * HD + ds * 512:
                                              sb * HD + (ds + 1) * 512])

            # ======== Phase B+C+D: attention (query-strip outer), output ====
            # Query strips t are the outer loop. After each strip's contexts
            # are normalized, the strip's ctxT (1MB bf16) is AllGathered
            # within the batch pair so both cores hold all 16 heads' context;
            # each core then projects its own 1024 output columns with no
            # cross-core reduction. The C chunk for strip t runs after the
            # attention of strip t+1, hiding the gather latency.
            with (
                tc.tile_pool(name="bctx", bufs=1) as cxp,
                tc.tile_pool(name="bmask", bufs=1) as mp,
                tc.tile_pool(name="bkv", bufs=4) as kvp,
                tc.tile_pool(name="bq", bufs=4) as bqp,
                tc.tile_pool(name="bex", bufs=6) as exp_,
                tc.tile_pool(name="bsm", bufs=2) as smp,
                tc.tile_pool(name="cga", bufs=2) as cga,
                tc.tile_pool(name="bps", bufs=3, space="PSUM") as pssp,
                tc.tile_pool(name="bpc", bufs=2, space="PSUM") as pscp,
                tc.tile_pool(name="bpm", bufs=2, space="PSUM") as psmp,
                tc.tile_pool(name="bpr", bufs=1, space="PSUM") as psrp,
                tc.tile_pool(name="co", bufs=4) as cop,
            ):
                ctx_sb = [cxp.tile([128, S], BF16, name=f"ctx{h}") for h in range(HL)]
                mask_sb = mp.tile([128, 128], F32R)
                nc.sync.dma_start(mask_sb[:], mask128[:])
                mask_bf = mp.tile([128, 128], BF16)
                nc.vector.tensor_copy(mask_bf[:], mask_sb[:])
                ones_j = mask_bf[:, 127:128]   # col 127: all ones (bf16)
                ones_b = mask_sb[0:1, 0:128]   # row 0: all ones (f32r)

                ctxs_d = [dram.tile([HD, 512], BF16, name=f"ctxs{i}")
                          for i in range(4)]
                # split gathers: half A (own heads 0-3) fires mid-strip and is
                # fully hidden; only half B (heads 4-7, 512KB) lands at strip
                # end. Rank order makes the block mapping core-independent:
                # ctxga rows = global heads [0-3 | 8-11], ctxgb = [4-7 | 12-15].
                ctxga_d = [dram.tile([HD, 512], BF16, name=f"ctxga{i}")
                           for i in range(4)]
                ctxgb_d = [dram.tile([HD, 512], BF16, name=f"ctxgb{i}")
                           for i in range(4)]

                # strip-deferred normalization state: (pc, sums, h, t)
                pending = []

                def flush_pending():
                    if not pending:
                        return
                    pcp_, recp_, hp_, tp2_ = pending.pop(0)
                    prb = psrp.tile([128, 512], F32,
                                    name=f"prb{hp_}{tp2_}", tag="prb")
                    nc.tensor.matmul(prb[:], ones_b, recp_[:],
                                     start=True, stop=True,
                                     skip_group_check=True)
                    rb = smp.tile([128, 512], F32, name=f"rb{hp_}{tp2_}", tag="rb")
                    nc.vector.tensor_copy(rb[:], prb[:])
                    nc.vector.tensor_mul(
                        ctx_sb[hp_][:, tp2_ * 512:(tp2_ + 1) * 512],
                        pcp_[:], rb[:])
                    # ship this head's ctx chunk right away so the strip-end
                    # gather only waits on the final head's store
                    nc.gpsimd.dma_start(
                        ctxs_d[tp2_][hp_ * 128:(hp_ + 1) * 128, :],
                        ctx_sb[hp_][:, tp2_ * 512:(tp2_ + 1) * 512])

                def emit_cout(tp_):
                    # project strip tp_ (rows [tp_*512, tp_*512+512)) onto this
                    # core's 1024 output columns, contracting all 16 heads of
                    # the gathered context.
                    ctxa = cga.tile([128, 16 * 512], BF16,
                                    name=f"cga{tp_}", tag="cga")
                    csrca = ctxga_d[tp_][:].rearrange("(a p) c -> p a c", p=128)
                    csrcb = ctxgb_d[tp_][:].rearrange("(a p) c -> p a c", p=128)
                    cdst = ctxa[:].rearrange("p (a c) -> p a c", c=512)
                    nc.gpsimd.dma_start(cdst[:, 0:4], csrca[:, 0:4])
                    nc.gpsimd.dma_start(cdst[:, 8:12], csrca[:, 4:8])
                    nc.sync.dma_start(cdst[:, 4:8], csrcb[:, 0:4])
                    nc.sync.dma_start(cdst[:, 12:16], csrcb[:, 4:8])
                    with nc.named_scope(f"C_out{tp_}"):
                        horder = [0, 1, 2, 3, 8, 9, 10, 11,
                                  4, 5, 6, 7, 12, 13, 14, 15]
                        for ms in range(2):
                            for sbl in range(4):
                                po = pssp.tile([128, 512], F32,
                                               name=f"po{tp_}{sbl}{ms}",
                                               tag="pss")
                                for hi, ht in enumerate(horder):
                                    nc.tensor.matmul(
                                        po[:],
                                        ctxa[:, ht * 512 + sbl * 128:
                                             ht * 512 + (sbl + 1) * 128],
                                        wos_all[:, ht * (M // 2) + ms * 512:
                                                ht * (M // 2) + (ms + 1) * 512],
                                        start=(hi == 0), stop=(hi == 2 * HL - 1))
                                ot = cop.tile([128, 512], F32,
                                              name=f"ot{tp_}{sbl}{ms}", tag="ot")
                                nc.vector.tensor_copy(ot[:], po[:])
                                r0 = tp_ * 512 + sbl * 128
                                yeng = nc.gpsimd if (sbl % 2 == 0) else nc.sync
                                yeng.dma_start(
                                    y[r0:r0 + 128, ms * 512:(ms + 1) * 512],
                                    ot[:])
                                if DEBUG:
                                    nc.scalar.dma_start(
                                        dbg["outp"][r0:r0 + 128,
                                                    ms * 512:(ms + 1) * 512],
                                        ot[:])

                def load_head(t2, h2):
                    # kro/vh/qr prefetch for head h2 of strip t2 (issued one
                    # head ahead so the first scores matmul never waits).
                    njt2 = 4 * t2 + 4
                    kro = kvp.tile([128, njt2 * 128], BF16,
                                   name=f"kro{h2}{t2}", tag="kro")
                    nc.sync.dma_start(
                        kro[:], krot_d[h2 * 128:(h2 + 1) * 128, 0:njt2 * 128])
                    qr2 = bqp.tile([128, 512], BF16, name=f"bq{h2}{t2}", tag="bq")
                    nc.sync.dma_start(qr2[:],
                                      qrot_d[h2 * 128:(h2 + 1) * 128,
                                             t2 * 512:(t2 + 1) * 512])
                    return kro, qr2

                # C chunks scheduled as late as possible: gather latency is
                # ~50-90us, so each strip's gather gets at least a strip's
                # worth of attention before its C chunk runs.
                cout_at = {(2, 3): 0, (3, 1): 1, (3, 5): 2}
                seq = [(t, h) for t in range(4) for h in range(HL)]

                # final emit_back + denominator handoff of each head is
                # deferred into the next head's body, so the new head's first
                # exp (ACT) overlaps the PE tail of the previous head instead
                # of serializing with it.
                back_pend = []

                def run_tail():
                    while back_pend:
                        back_pend.pop(0)()

                with nc.named_scope("B_attn"):
                    nxt = load_head(0, 0)
                    for si, (t, h) in enumerate(seq):
                        njt = 4 * t + 4
                        if True:
                            kro, qr = nxt
                            if si + 1 < len(seq):
                                nxt = load_head(seq[si + 1][0], seq[si + 1][1])
                            # flush the oldest pending head here: its
                            # reciprocal has had ~1.5 heads to finish, and
                            # flushing before the pc/pm allocations below
                            # keeps at most two accumulators live per bank.
                            flush_pending()
                            if (t, h) in cout_at:
                                emit_cout(cout_at[(t, h)])
                            if h == 5:
                                # heads 0-3 of this strip are flushed and
                                # stored by now; gather half A early.
                                nc.gpsimd.collective_compute(
                                    "AllGather", mybir.AluOpType.bypass,
                                    replica_groups=[[0, 1], [2, 3],
                                                    [4, 5], [6, 7]],
                                    ins=[ctxs_d[t][0:4 * 128, :]],
                                    outs=[ctxga_d[t][:]])
                            if h == 2 and t > 0:
                                # previous strip's heads 6,7 flushed during
                                # h0/h1 of this strip; its half-B gather can
                                # go now (strip 3's fires at its own end).
                                nc.gpsimd.collective_compute(
                                    "AllGather", mybir.AluOpType.bypass,
                                    replica_groups=[[0, 1], [2, 3],
                                                    [4, 5], [6, 7]],
                                    ins=[ctxs_d[t - 1][4 * 128:HD, :]],
                                    outs=[ctxgb_d[t - 1][:]])
                            pc = pscp.tile([128, 512], F32, name=f"pc{h}{t}", tag="pc")
                            pm = psmp.tile([1, 512], F32, name=f"pm{h}{t}", tag="pm")
                            exs = []

                            def emit_front(jt):
                                # scoresT block + exp into SBUF (+ diagonal mask)
                                cut = 128 * (jt - 4 * t) if jt >= 4 * t else 0
                                pss = pssp.tile([128, 512], F32,
                                                name=f"pss{h}{t}{jt}", tag="pss")
                                nc.tensor.matmul(pss[:, cut:512],
                                                 kro[:, jt * 128:(jt + 1) * 128],
                                                 qr[:, cut:512],
                                                 start=True, stop=True,
                                                 skip_group_check=True)
                                ex = exp_.tile([128, 512], BF16,
                                               name=f"ex{h}{t}{jt}", tag="ex")
                                nc.scalar.activation(
                                    ex[:, cut:512], pss[:, cut:512],
                                    mybir.ActivationFunctionType.Exp, scale=SCALE)
                                if jt >= 4 * t:
                                    nc.vector.tensor_mul(
                                        ex[:, cut:cut + 128],
                                        ex[:, cut:cut + 128], mask_bf[:])
                                exs.append((ex, cut))

                            def emit_back(jt, exs=exs, pm=pm, pc=pc,
                                          njt=njt, h=h):
                                # default-arg binding: the strip-final call is
                                # deferred into the next head's body, which
                                # rebinds these loop names.
                                ex, cut = exs[jt]
                                nc.tensor.matmul(pm[:, cut:512], ones_j,
                                                 ex[:, cut:512],
                                                 start=(jt == 0), stop=(jt == njt - 1),
                                                 skip_group_check=True)
                                nc.tensor.matmul(pc[:, cut:512],
                                                 v_res[:, jt * HD + h * 128:
                                                       jt * HD + (h + 1) * 128],
                                                 ex[:, cut:512],
                                                 start=(jt == 0), stop=(jt == njt - 1),
                                                 skip_group_check=True)

                            emit_front(0)
                            run_tail()
                            for jt in range(1, njt):
                                emit_front(jt)
                                emit_back(jt - 1)

                            def head_tail(pc=pc, pm=pm, h=h, t=t, njt=njt,
                                          emit_back=emit_back):
                                emit_back(njt - 1)
                                sums = smp.tile([1, 512], F32R,
                                                name=f"sums{h}{t}", tag="sums")
                                nc.vector.tensor_copy(sums[:], pm[:])
                                rec = smp.tile([1, 512], F32R,
                                               name=f"rec{h}{t}", tag="rec")
                                nc.vector.reciprocal(rec[:], sums[:])
                                pending.append((pc, rec, h, t))

                            back_pend.append(head_tail)

                        # ---- at strip end: ship this strip's ctx to the
                        # pair peer (the C chunk runs later, once the gather
                        # has had time to land).
                        if h == HL - 1 and t == 3:
                            run_tail()
                            while pending:
                                flush_pending()
                            nc.gpsimd.collective_compute(
                                "AllGather", mybir.AluOpType.bypass,
                                replica_groups=[[0, 1], [2, 3], [4, 5], [6, 7]],
                                ins=[ctxs_d[t][4 * 128:HD, :]],
                                outs=[ctxgb_d[t][:]])
                    emit_cout(3)

                if DEBUG:
                    for h in range(HL):
                        nc.sync.dma_start(dbg["ctxT"][h * 128:(h + 1) * 128, :],
                                          ctx_sb[h][:])

    _split_excess_waits(nc)
    return nc


# ---------------------------------------------------------------------------
# Host-side input prep / sharding
# ---------------------------------------------------------------------------

def _rope_tables():
    half = D // 2
    fraction = 2.0 * np.arange(half, dtype=np.float64) / D
    ts = MIN_WINDOW * (MAX_WINDOW / MIN_WINDOW) ** fraction
    ts = np.repeat(ts, 2)                              # [D]
    pos = np.arange(S, dtype=np.float64)
    sinusoid = pos[None, :] / ts[:, None]              # [D, S]
    cos = np.cos(sinusoid).astype(np.float32)
    sign = np.where(np.arange(D) % 2 == 1, 1.0, -1.0)
    sin = (np.sin(sinusoid) * sign[:, None]).astype(np.float32)
    return cos, sin


def _mask128():
    jj = np.arange(128)[:, None]
    ii = np.arange(128)[None, :]
    return (jj <= ii).astype(np.float32)


def _pmat():
    p = np.zeros((D, D), dtype=np.float32)
    idx = np.arange(D)
    p[idx, idx ^ 1] = 1.0
    return p


_CACHED = {}


def kernel(x, Wqkv, Wo):
    x = np.asarray(x, dtype=np.float32)
    Wqkv = np.asarray(Wqkv, dtype=np.float32)
    Wo = np.asarray(Wo, dtype=np.float32)

    cos, sin = _rope_tables()
    m128 = _mask128()
    pm = _pmat()

    bf16 = ml_dtypes.bfloat16
    in_maps = []
    for c in range(8):
        b, g = c // 2, c % 2
        hs = slice(g * HL, (g + 1) * HL)
        in_maps.append({
            "xt": np.ascontiguousarray(x[b].T).astype(bf16),
            "wq": np.ascontiguousarray(Wqkv[:, 0, hs, :].reshape(M, HD)).astype(bf16),
            "wk": np.ascontiguousarray(Wqkv[:, 1, hs, :].reshape(M, HD)).astype(bf16),
            "wv": np.ascontiguousarray(Wqkv[:, 2, hs, :].reshape(M, HD)).astype(bf16),
            "wo": np.ascontiguousarray(Wo[:, g * (M // 2):(g + 1) * (M // 2)]).astype(bf16),
            "cosT": cos, "sinT": sin, "pmat": pm.astype(bf16), "mask128": m128,
        })

    if "nc" not in _CACHED:
        _CACHED["nc"] = build_kernel()
    nc = _CACHED["nc"]

    res = run_bass_kernel_spmd(nc, in_maps, core_ids=list(range(8)),
                               trace=os.environ.get("MHA_KERNEL_TRACE", "0") == "1")
    _CACHED["last_results"] = res

    out = np.empty((B, S, M), dtype=np.float32)
    for c in range(8):
        b, g = c // 2, c % 2
        out[b, :, g * (M // 2):(g + 1) * (M // 2)] = res.results[c]["y"]
    return out


if __name__ == "__main__":
    rng = np.random.default_rng(0)
    x = rng.standard_normal((B, S, M), dtype=np.float32)
    Wqkv = (rng.standard_normal((M, 3, H, D), dtype=np.float32) / math.sqrt(M)).astype(np.float32)
    Wo = (rng.standard_normal((H * D, M), dtype=np.float32) / math.sqrt(H * D)).astype(np.float32)
    out = kernel(x=x, Wqkv=Wqkv, Wo=Wo)
    print("kernel ran, out shape", out.shape, "mean", float(np.abs(out).mean()))



# revision 54
# speedup vs baseline: 1.1154x; 1.1154x over previous
"""Trainium2 Bass kernel for nn_MultiHeadAttention_41455024341166.

Reference computation (B=4, S=2048, M=2048, H=16, D=128, fp32):
    qkv = einsum('bsm,mthd->bsthd', x, Wqkv); q,k,v = qkv[:,:,0..2]
    q,k = rope_consecutive(q), rope_consecutive(k)
    ctx = causal_softmax(q @ k^T / sqrt(D)) @ v   (per b,h)
    out = ctx.reshape(B,S,H*D) @ Wo

Sharding: 8 cores = 4 batches x 2 head-groups (core c -> b=c//2, g=c%2,
heads [8g, 8g+8)). Attention is fully head-parallel. For the output
projection each core owns M-columns [g*1024,(g+1)*1024): after each query
strip is normalized, the strip's ctxT (1MB bf16) is AllGathered within the
batch pair so both cores hold all 16 heads' context and project their own
column half with no cross-core reduction (4MB wire per core total, ~20us
per gather, overlapped with later strips' attention).

Kernel strategy (per core; all matmul operands bf16 - fp32r is full-rate
on the PE but its full-width multiplies draw enough power to trip the
activity throttle (50% util cap); bf16 runs measurably cooler. PSUM
accumulation is fp32 throughout; rel err vs the fp32 reference ~6e-3):
  A:  xT resident in SBUF once (per-block DMAs so compute starts early).
      A-qk: qT,kT = W^T-stationary @ xT-moving -> [d, s] layout; RoPE via a
            pair-swap permutation matmul + elementwise cos/sin tables, with
            the RoPE tail software-pipelined one tile behind the projection
            so the PE never waits on the scalar-engine PSUM evacuation.
      A-v:  v = xT-stationary @ Wv-moving -> [s, d], evacuated straight
            into a persistent SBUF tile (v never touches DRAM).
  B:  per head, per 512-query strip, two passes, loads prefetched one head
      ahead:
      pass1: scoresT[j,i] = krotT_j-stationary @ qrotT-moving (transposed
             scores - no prob transpose needed), diagonal blocks sliced to
             the causal region; exp fused into the PSUM evacuation (no max
             subtraction; scores are O(5) here); causal mask =
             multiplicative 0/1 mask after exp (on DVE); softmax
             denominators accumulate via ones-vector matmuls.
      pass2: ctxT += v_j-stationary @ expT-moving; each head's final
             accumulation step + denominator handoff is deferred into the
             next head's body so the new head's first exp overlaps it. The
             [1,512] reciprocal (3.3us, single-lane on DVE) runs early and
             the normalization (a K=1 ones broadcast matmul + DVE mul) is
             deferred ~4 score blocks so it never stalls the PE.
  C:  per strip, after its pairwise ctx AllGather lands: own output
      columns = gathered-ctxT-stationary @ Wo-moving contracted over all
      16 heads; Wo (4MB bf16, all heads x own columns) is SBUF-resident
      from kernel start. Each strip's gather is split in halves (heads 0-3
      fire mid-strip, heads 4-7 ride the next strip, except strip 3's at
      its end) and C chunks are scheduled mid-later-strips, so only the
      last strip's half-gather + C chunk (~40us) is exposed at the tail.
      DMA descriptor writes ride the otherwise-idle gpsimd/sync queues --
      a dma_start occupies its issuing engine ~0.6-2us, which would starve
      the ACT exp pipeline (B's pacer) or the evacuations in A.
"""

import os
import sys
import types
import math

import ml_dtypes
import numpy as np

import concourse.bass as bass
import concourse.tile as tile
import concourse.mybir as mybir
from concourse.bass_utils import run_bass_kernel_spmd

F32 = mybir.dt.float32
F32R = mybir.dt.float32r
BF16 = mybir.dt.bfloat16

B, S, M, H, D = 4, 2048, 2048, 16, 128
HL = H // 2              # heads per core
HD = HL * D              # 1024
SCALE = 1.0 / math.sqrt(D)
MIN_WINDOW, MAX_WINDOW = 1.0, 10000.0

DEBUG = os.environ.get("MHA_KERNEL_DEBUG", "0") == "1"


# ---------------------------------------------------------------------------
# Workarounds for the trimmed walrus/axon stack in this container.
# ---------------------------------------------------------------------------

_WSPLIT_N = [0]


def _split_excess_waits(nc):
    """walrus here rejects instructions carrying more sync-waits than slots
    (1; EventSemaphore: 2). Hoist excess waits onto EventSemaphore carriers
    inserted before the offender on the same engine stream. Safe: Tile emits
    one linearized order where every wait's producer precedes its consumer."""
    for fn in nc.m.functions:
        for bb in fn.blocks:
            changed = False
            new_list = []
            for inst in bb.instructions:
                si = inst.sync_info
                waits = list(si.on_wait) if si is not None else []
                cap = 2 if isinstance(inst, mybir.InstEventSemaphore) else 1
                if len(waits) > cap:
                    keep, excess = waits[-cap:], waits[:-cap]
                    for i in range(0, len(excess), 2):
                        _WSPLIT_N[0] += 1
                        new_list.append(mybir.InstEventSemaphore(
                            name=f"wsplit-{_WSPLIT_N[0]}", ins=[], outs=[],
                            engine=inst.engine,
                            sync_info=mybir.SyncInfo(on_wait=excess[i:i + 2],
                                                     on_update=[])))
                    si.on_wait = keep
                    changed = True
                new_list.append(inst)
            if changed:
                bb.instructions = new_list


def _register_ntff_hook():
    """antenv.axon_hooks is absent in this image, so boot skipped registering
    the NTFF profiling hook; recreate it so trace=True works."""
    if "antenv.axon_hooks" in sys.modules:
        return
    try:
        import antenv as _antenv
        m = types.ModuleType("antenv.axon_hooks")
        m._hook = None
        m.set_axon_ntff_profile_hook = lambda h, _m=m: setattr(_m, "_hook", h)
        m.get_axon_ntff_profile_hook = lambda _m=m: _m._hook
        sys.modules["antenv.axon_hooks"] = m
        _antenv.axon_hooks = m
        from trn_agent_boot.trn_boot import _ntff_profile_via_ctypes
        m.set_axon_ntff_profile_hook(
            _ntff_profile_via_ctypes('/opt/axon/libaxon_pjrt.so'))
    except Exception:
        pass


_register_ntff_hook()


# ---------------------------------------------------------------------------
# Kernel builder (per-core SPMD program)
# ---------------------------------------------------------------------------

def _blocked_dma(eng, dst_ap, dram_full, c0, c1, nrows=None):
    """One DMA moving cols [c0,c1) (and optionally only the first nrows rows)
    of a [R, C] DRAM tensor into a [128, (nrows//128)*(c1-c0)] SBUF tile whose
    column block a holds source rows [a*128, (a+1)*128)."""
    src = dram_full.rearrange("(a p) c -> p a c", p=128)
    if nrows is not None:
        src = src[:, 0:nrows // 128, :]
    src = src[:, :, c0:c1]
    dst = dst_ap.rearrange("p (a c) -> p a c", c=c1 - c0)
    eng.dma_start(dst, src)


def build_kernel():
    nc = bass.Bass("TRN2", target_bir_lowering=False, num_devices=8)

    xt = nc.dram_tensor("xt", [M, S], BF16, kind="ExternalInput")       # x[b].T
    wq = nc.dram_tensor("wq", [M, HD], BF16, kind="ExternalInput")
    wk = nc.dram_tensor("wk", [M, HD], BF16, kind="ExternalInput")
    wv = nc.dram_tensor("wv", [M, HD], BF16, kind="ExternalInput")
    wo = nc.dram_tensor("wo", [2 * HD, M // 2], BF16, kind="ExternalInput")
    cosT = nc.dram_tensor("cosT", [D, S], F32, kind="ExternalInput")
    sinT = nc.dram_tensor("sinT", [D, S], F32, kind="ExternalInput")    # sign-folded
    pmat = nc.dram_tensor("pmat", [D, D], BF16, kind="ExternalInput")   # adjacent-pair swap
    mask128 = nc.dram_tensor("mask128", [128, 128], F32R, kind="ExternalInput")
    # Output columns: this core owns M-columns [g*1024, (g+1)*1024) of out[b].
    # The per-strip ctx AllGather (pairwise) gives each core all 16 heads'
    # context, so its column block needs no cross-core reduction.
    y = nc.dram_tensor("y", [S, M // 2], F32, kind="ExternalOutput")

    dbg = {}
    if DEBUG:
        dbg["qrot"] = nc.dram_tensor("dbg_qrot", [HD, S], BF16, kind="ExternalOutput")
        dbg["krot"] = nc.dram_tensor("dbg_krot", [HD, S], BF16, kind="ExternalOutput")
        dbg["v"] = nc.dram_tensor("dbg_v", [S, HD], BF16, kind="ExternalOutput")
        dbg["ctxT"] = nc.dram_tensor("dbg_ctxT", [HD, S], BF16, kind="ExternalOutput")
        dbg["outp"] = nc.dram_tensor("dbg_outp", [S, M // 2], F32, kind="ExternalOutput")

    with nc.allow_low_precision(reason="fp32r matmul kernel"), \
         tile.TileContext(nc) as tc:
        with tc.tile_pool(name="dram", bufs=1, space="DRAM") as dram, \
             tc.tile_pool(name="wo_res", bufs=1) as wrp:
            qrot_d = dram.tile([HD, S], BF16)
            krot_d = dram.tile([HD, S], BF16)

            # v stays in SBUF end-to-end: block sb of the free axis holds
            # v[sb*128:(sb+1)*128, :] i.e. rows (keys) x all 1024 hd cols, so
            # the PV stationary for (head h, key block jt) is
            # v_res[:, jt*1024 + h*128 : jt*1024 + (h+1)*128].
            v_res = wrp.tile([128, 16 * HD], BF16)

            warm_in = dram.tile([1, 128], F32, name="warm_in")
            warm_out = dram.tile([2, 128], F32, name="warm_out")

            # Wo resident in SBUF for the whole kernel: wo holds all 16 heads'
            # rows x this core's 1024 output columns. Block a of the free axis
            # holds rows [a*128,(a+1)*128) (= global head a), so the C-phase
            # moving operand for head ht / col-strip ms is
            # wos_all[:, ht*1024 + ms*512 : ht*1024 + (ms+1)*512].
            wos_all = wrp.tile([128, 16 * (M // 2)], BF16)
            _blocked_dma(nc.scalar, wos_all[:], wo[:], 0, M // 2)

            # ======== Phase A: projections off one resident xT ========
            # xT lives in 16 per-mt tiles so the first projection matmuls can
            # start as soon as the first 1MB row-block lands.
            with tc.tile_pool(name="ax", bufs=1) as xp, \
                 tc.tile_pool(name="avw", bufs=1) as wvp:
                xts = []


                # ---- A-qk: qT,kT + RoPE ----
                # one projection PSUM pool spans A-qk and A-v (same tag), so
                # the bank rotation continues across the phase boundary
                # instead of opening a fresh pool whose banks carry WARs
                # against the old pool's last readers.
                apsp_cm = tc.tile_pool(name="aps", bufs=3, space="PSUM")
                apsp = apsp_cm.__enter__()
                with nc.named_scope("A_qk"):
                    with (
                        tc.tile_pool(name="atab", bufs=1) as tabp,
                        tc.tile_pool(name="aw", bufs=3) as wp,
                        tc.tile_pool(name="at", bufs=3) as tp,
                        tc.tile_pool(name="aps2", bufs=2, space="PSUM") as psp2,
                    ):
                        psp = apsp
                        cos_sb = tabp.tile([128, S], F32)
                        nc.gpsimd.dma_start(cos_sb[:], cosT[:])
                        sin_sb = tabp.tile([128, S], F32)
                        nc.gpsimd.dma_start(sin_sb[:], sinT[:])
                        p_sb = tabp.tile([128, 128], BF16)
                        nc.gpsimd.dma_start(p_sb[:], pmat[:])
                        # warm up the CC path early so the first real gather
                        # doesn't pay the handshake cost (queued behind the
                        # small table loads above).
                        nc.gpsimd.collective_compute(
                            "AllGather", mybir.AluOpType.bypass,
                            replica_groups=[[0, 1], [2, 3], [4, 5], [6, 7]],
                            ins=[warm_in[:]], outs=[warm_out[:]])
                        # first weight blocks go ahead of the 16MB xT load so
                        # the projection can start as soon as quarter 0 lands
                        wblk_pre = {}
                        for h0, qk0, wt0 in ((0, 0, wq), (0, 1, wk), (1, 0, wq)):
                            wb = wp.tile([128, 16 * 128], BF16,
                                         name=f"wblk{h0}{qk0}", tag="wblk")
                            _blocked_dma(nc.sync, wb[:], wt0[:],
                                         h0 * 128, (h0 + 1) * 128)
                            wblk_pre[(h0, qk0)] = wb
                        for q4 in range(4):
                            xti = xp.tile([128, 4 * S], BF16, name=f"xt{q4}")
                            # per-block DMAs so the first projection chain can
                            # start as soon as the first 512KB lands
                            for a in range(4):
                                nc.sync.dma_start(
                                    xti[:, a * S:(a + 1) * S],
                                    xt.rearrange("(a p) c -> p a c", p=128)
                                      [:, q4 * 4 + a, :])
                            xts.append(xti)
                        wvs0 = wvp.tile([128, 16 * 512], BF16,
                                        name="wvs0", tag="wvs")
                        _blocked_dma(nc.sync, wvs0[:], wv[:], 0, 512)

                        # RoPE tail (pmat matmul + cos/sin fold) runs one tile
                        # behind the projection chain so the PE never waits on
                        # the scalar-engine PSUM evacuation it consumes.
                        rope_pend = []

                        def rope_tail():
                            if not rope_pend:
                                return
                            q_sb, h, qk, t, outd = rope_pend.pop()
                            ps2 = psp2.tile([128, 512], F32,
                                            name=f"psw{h}{qk}{t}", tag="psw")
                            nc.tensor.matmul(ps2[:], p_sb[:], q_sb[:],
                                             start=True, stop=True)
                            t2 = tp.tile([128, 512], F32,
                                         name=f"t2{h}{qk}{t}", tag="t2")
                            nc.vector.tensor_mul(t2[:], ps2[:],
                                                 sin_sb[:, t * 512:(t + 1) * 512])
                            t1 = tp.tile([128, 512], F32,
                                         name=f"t1{h}{qk}{t}", tag="t1")
                            nc.vector.tensor_mul(t1[:], q_sb[:],
                                                 cos_sb[:, t * 512:(t + 1) * 512])
                            qr = tp.tile([128, 512], BF16,
                                         name=f"qr{h}{qk}{t}", tag="qr")
                            nc.vector.tensor_add(qr[:], t1[:], t2[:])
                            nc.gpsimd.dma_start(
                                outd[h * 128:(h + 1) * 128,
                                     t * 512:(t + 1) * 512], qr[:])
                            if DEBUG:
                                nc.sync.dma_start(
                                    dbg["qrot" if qk == 0 else "krot"]
                                    [h * 128:(h + 1) * 128,
                                     t * 512:(t + 1) * 512], qr[:])

                        groups = [(h, qk) for h in range(HL) for qk in (0, 1)]
                        wtab = {0: (wq, qrot_d), 1: (wk, krot_d)}

                        def fetch_wblk(h, qk):
                            if (h, qk) in wblk_pre:
                                return wblk_pre.pop((h, qk))
                            wblk = wp.tile([128, 16 * 128], BF16,
                                           name=f"wblk{h}{qk}", tag="wblk")
                            _blocked_dma(nc.sync, wblk[:], wtab[qk][0][:],
                                         h * 128, (h + 1) * 128)
                            return wblk

                        wnext = fetch_wblk(*groups[0])
                        for gi, (h, qk) in enumerate(groups):
                            wblk = wnext
                            if gi + 1 < len(groups):
                                wnext = fetch_wblk(*groups[gi + 1])
                            outd = wtab[qk][1]
                            if True:
                                for t in range(4):
                                    ps = psp.tile([128, 512], F32,
                                                  name=f"psq{h}{qk}{t}", tag="psq")
                                    for mt in range(16):
                                        nc.tensor.matmul(
                                            ps[:],
                                            wblk[:, mt * 128:(mt + 1) * 128],
                                            xts[mt // 4][:, (mt % 4) * S + t * 512:(mt % 4) * S + (t + 1) * 512],
                                            start=(mt == 0), stop=(mt == 15))
                                    q_sb = tp.tile([128, 512], BF16,
                                                   name=f"q{h}{qk}{t}", tag="q")
                                    nc.scalar.copy(q_sb[:], ps[:])
                                    rope_tail()
                                    rope_pend.append((q_sb, h, qk, t, outd))
                        rope_tail()

                # ---- A-v: v = x @ Wv (natural [s, hd]) ----
                with nc.named_scope("A_v"):
                    if True:
                        psp = apsp
                        for ds in range(2):
                            wvs = wvs0 if ds == 0 else wvp.tile(
                                [128, 16 * 512], BF16, name=f"wvs{ds}",
                                tag="wvs")
                            if ds != 0:
                                _blocked_dma(nc.sync, wvs[:], wv[:], ds * 512,
                                             (ds + 1) * 512)
                            for sb in range(16):
                                ps = psp.tile([128, 512], F32,
                                              name=f"psv{ds}{sb}", tag="psq")
                                for mt in range(16):
                                    nc.tensor.matmul(
                                        ps[:],
                                        xts[mt // 4][:, (mt % 4) * S + sb * 128:(mt % 4) * S + (sb + 1) * 128],
                                        wvs[:, mt * 512:(mt + 1) * 512],
                                        start=(mt == 0), stop=(mt == 15))
                                nc.scalar.copy(
                                    v_res[:, sb * HD + ds * 512:
                                          sb * HD + (ds + 1) * 512], ps[:])
                                if DEBUG:
                                    nc.sync.dma_start(
                                        dbg["v"][sb * 128:(sb + 1) * 128,
                                                 ds * 512:(ds + 1) * 512],
                                        v_res[:, sb * HD + ds * 512:
                                              sb * HD + (ds + 1) * 512])
                apsp_cm.__exit__(None, None, None)

            # ======== Phase B+C+D: attention (query-strip outer), output ====
            # Query strips t are the outer loop. After each strip's contexts
            # are normalized, the strip's ctxT (1MB bf16) is AllGathered
            # within the batch pair so both cores hold all 16 heads' context;
            # each core then projects its own 1024 output columns with no
            # cross-core reduction. The C chunk for strip t runs after the
            # attention of strip t+1, hiding the gather latency.
            with (
                tc.tile_pool(name="bctx", bufs=1) as cxp,
                tc.tile_pool(name="bmask", bufs=1) as mp,
                tc.tile_pool(name="bkv", bufs=4) as kvp,
                tc.tile_pool(name="bq", bufs=4) as bqp,
                tc.tile_pool(name="bex", bufs=6) as exp_,
                tc.tile_pool(name="bsm", bufs=2) as smp,
                tc.tile_pool(name="cga", bufs=2) as cga,
                tc.tile_pool(name="bps", bufs=3, space="PSUM") as pssp,
                tc.tile_pool(name="bpc", bufs=2, space="PSUM") as pscp,
                tc.tile_pool(name="bpm", bufs=2, space="PSUM") as psmp,
                tc.tile_pool(name="bpr", bufs=1, space="PSUM") as psrp,
                tc.tile_pool(name="co", bufs=4) as cop,
            ):
                ctx_sb = [cxp.tile([128, S], BF16, name=f"ctx{h}") for h in range(HL)]
                mask_sb = mp.tile([128, 128], F32R)
                nc.sync.dma_start(mask_sb[:], mask128[:])
                mask_bf = mp.tile([128, 128], BF16)
                nc.vector.tensor_copy(mask_bf[:], mask_sb[:])
                ones_j = mask_bf[:, 127:128]   # col 127: all ones (bf16)
                ones_b = mask_sb[0:1, 0:128]   # row 0: all ones (f32r)

                ctxs_d = [dram.tile([HD, 512], BF16, name=f"ctxs{i}")
                          for i in range(4)]
                # split gathers: half A (own heads 0-3) fires mid-strip and is
                # fully hidden; only half B (heads 4-7, 512KB) lands at strip
                # end. Rank order makes the block mapping core-independent:
                # ctxga rows = global heads [0-3 | 8-11], ctxgb = [4-7 | 12-15].
                ctxga_d = [dram.tile([HD, 512], BF16, name=f"ctxga{i}")
                           for i in range(4)]
                ctxgb_d = [dram.tile([HD, 512], BF16, name=f"ctxgb{i}")
                           for i in range(4)]

                # strip-deferred normalization state: (pc, sums, h, t)
                pending = []

                def flush_pending():
                    if not pending:
                        return
                    pcp_, recp_, hp_, tp2_ = pending.pop(0)
                    prb = psrp.tile([128, 512], F32,
                                    name=f"prb{hp_}{tp2_}", tag="prb")
                    nc.tensor.matmul(prb[:], ones_b, recp_[:],
                                     start=True, stop=True,
                                     skip_group_check=True)
                    rb = smp.tile([128, 512], F32, name=f"rb{hp_}{tp2_}", tag="rb")
                    nc.vector.tensor_copy(rb[:], prb[:])
                    nc.vector.tensor_mul(
                        ctx_sb[hp_][:, tp2_ * 512:(tp2_ + 1) * 512],
                        pcp_[:], rb[:])
                    # ship this head's ctx chunk right away so the strip-end
                    # gather only waits on the final head's store
                    nc.gpsimd.dma_start(
                        ctxs_d[tp2_][hp_ * 128:(hp_ + 1) * 128, :],
                        ctx_sb[hp_][:, tp2_ * 512:(tp2_ + 1) * 512])

                def emit_cout(tp_):
                    # project strip tp_ (rows [tp_*512, tp_*512+512)) onto this
                    # core's 1024 output columns, contracting all 16 heads of
                    # the gathered context.
                    ctxa = cga.tile([128, 16 * 512], BF16,
                                    name=f"cga{tp_}", tag="cga")
                    csrca = ctxga_d[tp_][:].rearrange("(a p) c -> p a c", p=128)
                    csrcb = ctxgb_d[tp_][:].rearrange("(a p) c -> p a c", p=128)
                    cdst = ctxa[:].rearrange("p (a c) -> p a c", c=512)
                    nc.gpsimd.dma_start(cdst[:, 0:4], csrca[:, 0:4])
                    nc.gpsimd.dma_start(cdst[:, 8:12], csrca[:, 4:8])
                    nc.sync.dma_start(cdst[:, 4:8], csrcb[:, 0:4])
                    nc.sync.dma_start(cdst[:, 12:16], csrcb[:, 4:8])
                    with nc.named_scope(f"C_out{tp_}"):
                        horder = [0, 1, 2, 3, 8, 9, 10, 11,
                                  4, 5, 6, 7, 12, 13, 14, 15]
                        for ms in range(2):
                            for sbl in range(4):
                                po = pssp.tile([128, 512], F32,
                                               name=f"po{tp_}{sbl}{ms}",
                                               tag="pss")
                                for hi, ht in enumerate(horder):
                                    nc.tensor.matmul(
                                        po[:],
                                        ctxa[:, ht * 512 + sbl * 128:
                                             ht * 512 + (sbl + 1) * 128],
                                        wos_all[:, ht * (M // 2) + ms * 512:
                                                ht * (M // 2) + (ms + 1) * 512],
                                        start=(hi == 0), stop=(hi == 2 * HL - 1))
                                ot = cop.tile([128, 512], F32,
                                              name=f"ot{tp_}{sbl}{ms}", tag="ot")
                                nc.vector.tensor_copy(ot[:], po[:])
                                r0 = tp_ * 512 + sbl * 128
                                yeng = nc.gpsimd if (sbl % 2 == 0) else nc.sync
                                yeng.dma_start(
                                    y[r0:r0 + 128, ms * 512:(ms + 1) * 512],
                                    ot[:])
                                if DEBUG:
                                    nc.scalar.dma_start(
                                        dbg["outp"][r0:r0 + 128,
                                                    ms * 512:(ms + 1) * 512],
                                        ot[:])

                def load_head(t2, h2):
                    # kro/vh/qr prefetch for head h2 of strip t2 (issued one
                    # head ahead so the first scores matmul never waits).
                    njt2 = 4 * t2 + 4
                    kro = kvp.tile([128, njt2 * 128], BF16,
                                   name=f"kro{h2}{t2}", tag="kro")
                    nc.sync.dma_start(
                        kro[:], krot_d[h2 * 128:(h2 + 1) * 128, 0:njt2 * 128])
                    qr2 = bqp.tile([128, 512], BF16, name=f"bq{h2}{t2}", tag="bq")
                    nc.sync.dma_start(qr2[:],
                                      qrot_d[h2 * 128:(h2 + 1) * 128,
                                             t2 * 512:(t2 + 1) * 512])
                    return kro, qr2

                # C chunks scheduled as late as possible: gather latency is
                # ~50-90us, so each strip's gather gets at least a strip's
                # worth of attention before its C chunk runs.
                cout_at = {(2, 3): 0, (3, 1): 1, (3, 5): 2}
                seq = [(t, h) for t in range(4) for h in range(HL)]

                # final emit_back + denominator handoff of each head is
                # deferred into the next head's body, so the new head's first
                # exp (ACT) overlaps the PE tail of the previous head instead
                # of serializing with it.
                back_pend = []

                def run_tail():
                    while back_pend:
                        back_pend.pop(0)()

                with nc.named_scope("B_attn"):
                    pend_loads = [load_head(*seq[0][0:2]),
                                  load_head(seq[1][0], seq[1][1])]
                    for si, (t, h) in enumerate(seq):
                        njt = 4 * t + 4
                        if True:
                            kro, qr = pend_loads.pop(0)
                            if si + 2 < len(seq):
                                pend_loads.append(
                                    load_head(seq[si + 2][0], seq[si + 2][1]))
                            # flush the oldest pending head here: its
                            # reciprocal has had ~1.5 heads to finish, and
                            # flushing before the pc/pm allocations below
                            # keeps at most two accumulators live per bank.
                            flush_pending()
                            if (t, h) in cout_at:
                                emit_cout(cout_at[(t, h)])
                            if h == 5:
                                # heads 0-3 of this strip are flushed and
                                # stored by now; gather half A early.
                                nc.gpsimd.collective_compute(
                                    "AllGather", mybir.AluOpType.bypass,
                                    replica_groups=[[0, 1], [2, 3],
                                                    [4, 5], [6, 7]],
                                    ins=[ctxs_d[t][0:4 * 128, :]],
                                    outs=[ctxga_d[t][:]])
                            if h == 2 and t > 0:
                                # previous strip's heads 6,7 flushed during
                                # h0/h1 of this strip; its half-B gather can
                                # go now (strip 3's fires at its own end).
                                nc.gpsimd.collective_compute(
                                    "AllGather", mybir.AluOpType.bypass,
                                    replica_groups=[[0, 1], [2, 3],
                                                    [4, 5], [6, 7]],
                                    ins=[ctxs_d[t - 1][4 * 128:HD, :]],
                                    outs=[ctxgb_d[t - 1][:]])
                            pc = pscp.tile([128, 512], F32, name=f"pc{h}{t}", tag="pc")
                            pm = psmp.tile([1, 512], F32, name=f"pm{h}{t}", tag="pm")
                            exs = []

                            def emit_front(jt):
                                # scoresT block + exp into SBUF (+ diagonal mask)
                                cut = 128 * (jt - 4 * t) if jt >= 4 * t else 0
                                pss = pssp.tile([128, 512], F32,
                                                name=f"pss{h}{t}{jt}", tag="pss")
                                nc.tensor.matmul(pss[:, cut:512],
                                                 kro[:, jt * 128:(jt + 1) * 128],
                                                 qr[:, cut:512],
                                                 start=True, stop=True,
                                                 skip_group_check=True)
                                ex = exp_.tile([128, 512], BF16,
                                               name=f"ex{h}{t}{jt}", tag="ex")
                                nc.scalar.activation(
                                    ex[:, cut:512], pss[:, cut:512],
                                    mybir.ActivationFunctionType.Exp, scale=SCALE)
                                if jt >= 4 * t:
                                    nc.gpsimd.tensor_mul(
                                        ex[:, cut:cut + 128],
                                        ex[:, cut:cut + 128], mask_bf[:])
                                exs.append((ex, cut))

                            def emit_back(jt, exs=exs, pm=pm, pc=pc,
                                          njt=njt, h=h):
                                # default-arg binding: the strip-final call is
                                # deferred into the next head's body, which
                                # rebinds these loop names.
                                ex, cut = exs[jt]
                                nc.tensor.matmul(pm[:, cut:512], ones_j,
                                                 ex[:, cut:512],
                                                 start=(jt == 0), stop=(jt == njt - 1),
                                                 skip_group_check=True)
                                nc.tensor.matmul(pc[:, cut:512],
                                                 v_res[:, jt * HD + h * 128:
                                                       jt * HD + (h + 1) * 128],
                                                 ex[:, cut:512],
                                                 start=(jt == 0), stop=(jt == njt - 1),
                                                 skip_group_check=True)

                            emit_front(0)
                            run_tail()
                            for jt in range(1, njt):
                                emit_front(jt)
                                emit_back(jt - 1)

                            def head_tail(pc=pc, pm=pm, h=h, t=t, njt=njt,
                                          emit_back=emit_back):
                                emit_back(njt - 1)
                                sums = smp.tile([1, 512], F32R,
                                                name=f"sums{h}{t}", tag="sums")
                                nc.vector.tensor_copy(sums[:], pm[:])
                                rec = smp.tile([1, 512], F32R,
                                               name=f"rec{h}{t}", tag="rec")
                                nc.vector.reciprocal(rec[:], sums[:])
                                pending.append((pc, rec, h, t))

                            back_pend.append(head_tail)

                        # ---- at strip end: ship this strip's ctx to the
                        # pair peer (the C chunk runs later, once the gather
                        # has had time to land).
                        if h == HL - 1 and t == 3:
                            run_tail()
                            while pending:
                                flush_pending()
                            nc.gpsimd.collective_compute(
                                "AllGather", mybir.AluOpType.bypass,
                                replica_groups=[[0, 1], [2, 3], [4, 5], [6, 7]],
                                ins=[ctxs_d[t][4 * 128:HD, :]],
                                outs=[ctxgb_d[t][:]])
                    emit_cout(3)

                if DEBUG:
                    for h in range(HL):
                        nc.sync.dma_start(dbg["ctxT"][h * 128:(h + 1) * 128, :],
                                          ctx_sb[h][:])

    _split_excess_waits(nc)
    return nc


# ---------------------------------------------------------------------------
# Host-side input prep / sharding
# ---------------------------------------------------------------------------

def _rope_tables():
    half = D // 2
    fraction = 2.0 * np.arange(half, dtype=np.float64) / D
    ts = MIN_WINDOW * (MAX_WINDOW / MIN_WINDOW) ** fraction
    ts = np.repeat(ts, 2)                              # [D]
    pos = np.arange(S, dtype=np.float64)
    sinusoid = pos[None, :] / ts[:, None]              # [D, S]
    cos = np.cos(sinusoid).astype(np.float32)
    sign = np.where(np.arange(D) % 2 == 1, 1.0, -1.0)
    sin = (np.sin(sinusoid) * sign[:, None]).astype(np.float32)
    return cos, sin


def _mask128():
    jj = np.arange(128)[:, None]
    ii = np.arange(128)[None, :]
    return (jj <= ii).astype(np.float32)


def _pmat():
    p = np.zeros((D, D), dtype=np.float32)
    idx = np.arange(D)
    p[idx, idx ^ 1] = 1.0
    return p


_CACHED = {}


def kernel(x, Wqkv, Wo):
    x = np.asarray(x, dtype=np.float32)
    Wqkv = np.asarray(Wqkv, dtype=np.float32)
    Wo = np.asarray(Wo, dtype=np.float32)

    cos, sin = _rope_tables()
    m128 = _mask128()
    pm = _pmat()

    bf16 = ml_dtypes.bfloat16
    in_maps = []
    for c in range(8):
        b, g = c // 2, c % 2
        hs = slice(g * HL, (g + 1) * HL)
        in_maps.append({
            "xt": np.ascontiguousarray(x[b].T).astype(bf16),
            "wq": np.ascontiguousarray(Wqkv[:, 0, hs, :].reshape(M, HD)).astype(bf16),
            "wk": np.ascontiguousarray(Wqkv[:, 1, hs, :].reshape(M, HD)).astype(bf16),
            "wv": np.ascontiguousarray(Wqkv[:, 2, hs, :].reshape(M, HD)).astype(bf16),
            "wo": np.ascontiguousarray(Wo[:, g * (M // 2):(g + 1) * (M // 2)]).astype(bf16),
            "cosT": cos, "sinT": sin, "pmat": pm.astype(bf16), "mask128": m128,
        })

    if "nc" not in _CACHED:
        _CACHED["nc"] = build_kernel()
    nc = _CACHED["nc"]

    res = run_bass_kernel_spmd(nc, in_maps, core_ids=list(range(8)),
                               trace=os.environ.get("MHA_KERNEL_TRACE", "0") == "1")
    _CACHED["last_results"] = res

    out = np.empty((B, S, M), dtype=np.float32)
    for c in range(8):
        b, g = c // 2, c % 2
        out[b, :, g * (M // 2):(g + 1) * (M // 2)] = res.results[c]["y"]
    return out


if __name__ == "__main__":
    rng = np.random.default_rng(0)
    x = rng.standard_normal((B, S, M), dtype=np.float32)
    Wqkv = (rng.standard_normal((M, 3, H, D), dtype=np.float32) / math.sqrt(M)).astype(np.float32)
    Wo = (rng.standard_normal((H * D, M), dtype=np.float32) / math.sqrt(H * D)).astype(np.float32)
    out = kernel(x=x, Wqkv=Wqkv, Wo=Wo)
    print("kernel ran, out shape", out.shape, "mean", float(np.abs(out).mean()))

